# revision 25
# baseline (speedup 1.0000x reference)
"""Trainium2 Bass kernel: AttentionWithFeedForward (dense transformer block).

Sharding: 8 cores = (batch b = c//4) x (seq chunk of 1024 tokens = c%4).
Each core redundantly computes K/V over its full batch (no collectives),
Q/attention/FFN only for its own 1024-token chunk. The host rotates the
token axis per core so the own chunk is always columns 0:1024 (attention
is invariant to key order), keeping the device program identical across
cores.

Layout: all activations transposed [d_model, tok] ("ptile" layout
[128, d/128, tok]); host pre-transposes x/y and pre-casts weights to bf16.
Matmuls bf16 with fp32 PSUM accumulation.

Attention is a per-head software pipeline: per key-tile kk the PE emits
2 score matmuls (contraction d_head=64 on half the partitions), ACT emits
one Exp (the only ACT function in the attention phases -- no activation
table reloads), and the AV matmuls for kk-1 are emitted one iteration
behind so the PE never queues behind the exp of the same kk. Score PSUM
is double-buffered (2 tiles x 2 banks) and the AV accumulator pool is
double-buffered (2 x 2 banks) so head h+1's pipeline starts while head
h's normalize (DVE reciprocal + DMA broadcast + multiply) drains off the
critical path. Softmax denominators come from a ones-column packed into
V: even heads use a 65-wide lhsT (AV rows 0:64, denom row 64), odd heads
a 128-wide shifted view (AV rows 64:128, denom row 32) keeping DVE lane
alignment for the normalize multiply.

LayerNorm rstd = reciprocal(sqrt(var+eps)): Sqrt on ACT (its own table,
loaded once per LN) + fast approximate reciprocal on DVE.

SBUF is a two-sided stack allocator: frees must be LIFO per side, so big
tensors are placed left/right in nested lifetime order.
"""

from contextlib import ExitStack

import numpy as np
import ml_dtypes

import concourse.bass as bass
import concourse.tile as tile
from concourse import bacc, mybir
from concourse.bass_utils import run_bass_kernel_spmd

BF16 = mybir.dt.bfloat16
F32 = mybir.dt.float32
FP8 = mybir.dt.float8e4
I32 = mybir.dt.int32
AF = mybir.ActivationFunctionType
OP = mybir.AluOpType
PM = mybir.MatmulPerfMode

P = 128
D = 512          # d_embed
EJ = D // P      # 4 ptiles
DC = 768         # d_cross
CJ = DC // P     # 6
FF = 2048
FJ = FF // P     # 16
H = 8
DH = 64
S = 4096
ST = S // P      # 32 key tiles (full batch)
CH = 1024        # tokens per core
N2 = CH // 512   # 2 free-dim slices
B = 2
NCORES = 8
EPS = 1e-5
GELU_AF = AF.Gelu_apprx_tanh
DEBUG = False       # adds intermediate DRAM dumps (dev only)
# Schraudolph exp on DVE for every DVE_EXP_MOD-th SA key tile (0 = off):
# exp(x) ~ bitcast_f32(int32(A*x + B)), ~3% elem error that largely
# cancels in the softmax ratio; offloads the saturated ACT engine.
DVE_EXP_MOD = 3
SCH_A = float(2 ** 23 / np.log(2))
SCH_B = float(127 * 2 ** 23 - 0.043677 * 2 ** 23)
INLINE_AV = False   # emit AV right after exp (no one-behind pipelining)
# Attention denominators sit at partition base 32/64 where
# reciprocal_approx_fast silently returns garbage (custom-DVE op only
# works at partition base 0); False routes them through a partition-0
# bounce for the fast approx, True uses bit-exact reciprocal in place.
EXACT_RECIP = False

# bias_cols column layout; column j of a param holds param[128*j + p].
_BC = {}
_c = 0
for _nm, _n in [("qb", 4), ("kb", 4), ("vb", 4), ("saob", 4), ("caqb", 4),
                ("cakb", 4), ("cavb", 4), ("caob", 4), ("ffb1", 16),
                ("ffb2", 4), ("ln1g", 4), ("ln1b", 4), ("ln2g", 4),
                ("ln2b", 4), ("ln3g", 4), ("ln3b", 4)]:
    _BC[_nm] = (_c, _n)
    _c += _n
NBC = _c


def _pt(a):
    """[din, N] -> [128, din//128, N] ptile layout (partition-inner)."""
    din, n = a.shape
    return np.ascontiguousarray(a.reshape(din // P, P, n).transpose(1, 0, 2))


def _bcol(v):
    """[din] -> [128, din//128]."""
    return np.ascontiguousarray(v.reshape(-1, P).T)


def _bcast_ap(row_ap, nparts):
    """Broadcast a [1, N] DRAM AP across nparts partitions (step 0)."""
    return bass.AP(tensor=row_ap.tensor, offset=row_ap.offset,
                   ap=[[0, nparts]] + [list(d) for d in row_ap.ap[1:]])


def build(ctx, tc, dram):
    """Emit the full per-core program. Returns (names, out_name)."""
    nc = tc.nc
    names = {}

    def din(key, shape, dtype):
        t = dram.tile(shape, dtype, kind="ExternalInput", name=f"i_{key}")
        names[key] = t.name
        return t

    # ---- DRAM I/O ----
    xt_bf_d = din("xt_bf", [P, EJ, S], BF16)     # x[b].T rotated, bf16
    xt_f32_d = din("xt_f32", [P, EJ, CH], F32)   # own chunk (cols 0:CH), f32
    yt_d = din("yt", [P, CJ, 77], BF16)          # y[b].T
    w_qkv_d = din("w_qkv", [P, EJ, 3 * D], BF16)
    w_sao_d = din("w_sao", [P, EJ, D], BF16)
    w_caq_d = din("w_caq", [P, EJ, D], BF16)
    w_cak_d = din("w_cak", [P, CJ, D], BF16)
    w_cav_d = din("w_cav", [P, CJ, D], BF16)
    w_cao_d = din("w_cao", [P, EJ, D], BF16)
    w_ff1_d = din("w_ff1", [P, EJ, FF], BF16)
    w_ff2_d = din("w_ff2", [P, FJ, D], BF16)
    bias_d = din("bias", [P, NBC], F32)
    out_d = dram.tile([P, EJ, CH], F32, kind="ExternalOutput", name="o_out")
    out_name = out_d.name

    dma = nc.sync.dma_start

    def sb(key, shape, dtype, side):
        return tc.tile(shape, dtype, name=f"s_{key}", side=side)

    def dump(key, t):
        if not DEBUG:
            return
        dt_ = dram.tile(list(t.shape), t.dtype, kind="ExternalOutput",
                        name=f"dbg_{key}")
        names[f"dbg_{key}"] = dt_.name
        ap = tuple([slice(None)] * len(t.shape))
        dma(out=dt_[ap], in_=t[ap])

    # ---- pools (never popped before build end) ----
    ps_s = ctx.enter_context(tc.tile_pool(name="ps_s", bufs=2, space="PSUM"))
    ps_o = ctx.enter_context(tc.tile_pool(name="ps_o", bufs=2, space="PSUM"))
    et_pool = ctx.enter_context(
        tc.tile_pool(name="et_pool", bufs=3, side="left"))
    eti_pool = ctx.enter_context(
        tc.tile_pool(name="eti_pool", bufs=2, side="left"))
    rep_pool = ctx.enter_context(
        tc.tile_pool(name="rep_pool", bufs=2, side="left"))
    dsc_pool = ctx.enter_context(
        tc.tile_pool(name="dsc_pool", bufs=4, space="DRAM"))

    # ---- permanent small tiles (right-side bottom) ----
    bias_t, free_bias = sb("bias", [P, NBC], F32, "right")
    dma(out=bias_t[:, :], in_=bias_d[:, :])

    def bc(nm, j):
        c0, _n = _BC[nm]
        return bias_t[:, c0 + j:c0 + j + 1]

    ones_col, free_ones = sb("ones_col", [P, 1], BF16, "right")
    nc.vector.memset(ones_col[:, :], 1.0)
    ones_f, free_ones_f = sb("ones_f", [P, 1], F32, "right")
    nc.vector.memset(ones_f[:, :], 1.0)
    eps_t, free_eps = sb("eps", [1, 1], F32, "right")
    nc.vector.memset(eps_t[:, :], EPS)
    yt, free_yt = sb("yt", [P, CJ, 77], BF16, "right")
    dma(out=yt[:, :, :], in_=yt_d[:, :, :])

    # ---- weights prefetched early (right side, kept to end) ----
    w_sao, free_w_sao = sb("w_sao", [P, EJ, D], BF16, "right")
    w_caq, free_w_caq = sb("w_caq", [P, EJ, D], BF16, "right")
    w_cak, free_w_cak = sb("w_cak", [P, CJ, D], BF16, "right")
    w_cav, free_w_cav = sb("w_cav", [P, CJ, D], BF16, "right")
    w_cao, free_w_cao = sb("w_cao", [P, EJ, D], BF16, "right")

    # ---- left stack: phase-1/SA tensors ----
    qt, free_qt = sb("qt", [P, EJ, CH], BF16, "left")
    kt, free_kt = sb("kt", [P, EJ, S], BF16, "left")
    v1, free_v1 = sb("v1", [P, ST, (H // 2) * 160], FP8, "left")
    xt_bf, free_xt_bf = sb("xt_bf", [P, EJ, S], BF16, "left")
    w_qkv, free_w_qkv = sb("w_qkv", [P, EJ, 3 * D], BF16, "left")
    dma(out=w_qkv[:, :, :], in_=w_qkv_d[:, :, :])
    for e in range(EJ):
        dma(out=xt_bf[:, e, :], in_=xt_bf_d[:, e, :])

    # residual-stream tensors
    xt_f32, free_xt_f32 = sb("xt_f32", [P, EJ, CH], F32, "right")
    dma(out=xt_f32[:, :, :], in_=xt_f32_d[:, :, :])
    ot, free_ot = sb("ot", [P, EJ, CH], BF16, "right")

    # remaining weight prefetches (behind the phase-1 inputs in the queue)
    dma(out=w_sao[:, :, :], in_=w_sao_d[:, :, :])
    dma(out=w_caq[:, :, :], in_=w_caq_d[:, :, :])
    dma(out=w_cak[:, :, :], in_=w_cak_d[:, :, :])
    dma(out=w_cav[:, :, :], in_=w_cav_d[:, :, :])
    dma(out=w_cao[:, :, :], in_=w_cao_d[:, :, :])

    v1h = v1[:, :, :].rearrange("p t (pr c) -> p t pr c", c=160)
    nc.vector.memset(v1h[:, :, :, 64:65], 1.0)
    nc.vector.memset(v1h[:, :, :, 65:96], 0.0)

    # ---- phase 1: QKV projections (transposed layout) ----
    # Loop order keeps the same lhsT for consecutive matmuls (weight reuse).
    for j in range(EJ):
        ps = ps_o.tile([P, 2, 512], F32, tag="po", name="ps_q")
        for e in range(EJ):
            for n in range(N2):
                nc.tensor.matmul(
                    ps[:, n, :], lhsT=w_qkv[:, e, P * j:P * (j + 1)],
                    rhs=xt_bf[:, e, 512 * n:512 * (n + 1)],
                    start=(e == 0), stop=(e == EJ - 1))
        nc.vector.tensor_scalar(
            out=qt[:, j, :],
            in0=ps[:, :, :].rearrange("p a b -> p (a b)"),
            scalar1=bc("qb", j), scalar2=None, op0=OP.add)
    for j in range(EJ):
        for g in range(S // CH):
            ps = ps_o.tile([P, 2, 512], F32, tag="po", name="ps_k")
            for e in range(EJ):
                for n in range(N2):
                    c0 = CH * g + 512 * n
                    nc.tensor.matmul(
                        ps[:, n, :], lhsT=w_qkv[:, e, D + P * j:D + P * (j + 1)],
                        rhs=xt_bf[:, e, c0:c0 + 512],
                        start=(e == 0), stop=(e == EJ - 1))
            nc.vector.tensor_scalar(
                out=kt[:, j, CH * g:CH * (g + 1)],
                in0=ps[:, :, :].rearrange("p a b -> p (a b)"),
                scalar1=bc("kb", j), scalar2=None, op0=OP.add)
    for t in range(ST):
        ps = ps_o.tile([P, 2, 512], F32, tag="po", name="ps_v")
        for e in range(EJ):
            nc.tensor.matmul(
                ps[:, 0, :], lhsT=xt_bf[:, e, P * t:P * (t + 1)],
                rhs=w_qkv[:, e, 2 * D:3 * D],
                start=(e == 0), stop=(e == EJ - 1))
        # V bias is applied after attention-normalize (per-partition there).
        # Both head-halves land in one strided copy: pair-block offsets
        # 0:64 (even) and 96:160 (odd) differ by a stride of 96.
        psh = ps[:, 0, :].rearrange("p (pr two c) -> p pr two c", two=2, c=64)
        v1t = v1[:, t, :]
        dst = bass.AP(tensor=v1t.tensor, offset=v1t.offset,
                      ap=[list(v1t.ap[0]), [160, H // 2], [96, 2], [1, 64]])
        nc.vector.tensor_copy(out=dst, in_=psh[:, :, :, :])
    dump("qt", qt)
    dump("kt", kt)
    dump("v1", v1)
    free_w_qkv()
    free_xt_bf()

    # ---- attention: per-head software pipeline ----
    def attn_head(h, kv_tiles, kp, kt_t, qt_t, v1_t, out_t, vb_nm):
        """One head: scores -> exp -> AV one iteration behind -> normalize.

        Even heads read AV rows 0:64 with the denominator at row 64; odd
        heads use the 128-wide shifted view of the packed [V|1] buffer so
        output lands on partitions 64:128 with the denominator at row 32.
        """
        jp, half = h // 2, h % 2
        dr = slice(DH * half, DH * (half + 1))
        o = ps_o.tile([P, 2, 512], F32, tag="po", name=f"o_h{half}")
        if half == 0:
            lhs_c0, om, d_row = 160 * jp, 65, 64
            orng = slice(0, 64)
        else:
            lhs_c0, om, d_row = 160 * jp + 32, 128, 32
            orng = slice(64, 128)

        def scores_exp(kk, et_dst):
            """Score matmuls for key-tile kk, then exp into et_dst
            ([kp, 2, 512] view, any dtype)."""
            sc = ps_s.tile([P, 2, 512], F32, tag="sc", name="sc")
            for n in range(N2):
                nc.tensor.matmul(
                    sc[0:kp, n, :],
                    lhsT=kt_t[dr, jp, P * kk:P * kk + kp],
                    rhs=qt_t[dr, jp, 512 * n:512 * (n + 1)],
                    start=True, stop=True)
            if DVE_EXP_MOD and kv_tiles > 1 and kk % DVE_EXP_MOD == (
                    DVE_EXP_MOD - 1):
                eti = eti_pool.tile([P, 2, 512], I32, tag="eti", name="eti")
                nc.vector.tensor_scalar(
                    out=eti[0:kp, :, :], in0=sc[0:kp, :, :],
                    scalar1=SCH_A * 0.125, scalar2=SCH_B,
                    op0=OP.mult, op1=OP.add)
                nc.vector.tensor_copy(out=et_dst,
                                      in_=eti[0:kp, :, :].bitcast(F32))
            else:
                nc.scalar.activation(et_dst, sc[0:kp, :, :], AF.Exp,
                                     scale=0.125)

        if kv_tiles > 1:
            # fp8 AV with DoubleRow: two key-tiles per matmul, et tiles
            # hold an exp PAIR [kp, kk-parity, n, 512] in fp8e4.
            npair = kv_tiles // 2

            def av8(pp, et):
                lhs = v1_t[0:kp, 2 * pp:2 * pp + 2, lhs_c0:lhs_c0 + om]
                for n in range(N2):
                    nc.tensor.matmul(o[0:om, n, :], lhsT=lhs,
                                     rhs=et[0:kp, :, n, :],
                                     start=(pp == 0), stop=(pp == npair - 1),
                                     perf_mode=PM.DoubleRow)

            prev = None
            for pp in range(npair):
                et = et_pool.tile([P, 2, 2, 512], FP8, tag="et", name="et")
                scores_exp(2 * pp, et[0:kp, 0, :, :])
                if prev is not None:
                    av8(pp - 1, prev)
                scores_exp(2 * pp + 1, et[0:kp, 1, :, :])
                prev = et
            av8(npair - 1, prev)
        else:
            def av(kk, et):
                lhs = v1_t[0:kp, kk, lhs_c0:lhs_c0 + om]
                for n in range(N2):
                    nc.tensor.matmul(o[0:om, n, :], lhsT=lhs,
                                     rhs=et[0:kp, n, :],
                                     start=(kk == 0),
                                     stop=(kk == kv_tiles - 1))

            for kk in range(kv_tiles):
                et = et_pool.tile([P, 2, 512], BF16, tag="et", name="et")
                scores_exp(kk, et[0:kp, :, :])
                av(kk, et)

        # normalize: rep rows = 1/denom broadcast, out = O*rep.
        # The denominator row sits at partition 32/64 where the fast
        # approximate reciprocal (custom DVE op) misbehaves, so: copy the
        # PSUM row to SBUF, DMA-shift it to a partition-0 tile, take the
        # fast reciprocal there, then bounce through DRAM to broadcast.
        rep = rep_pool.tile([P, 2, 512], F32, tag="rep", name="rep")
        if EXACT_RECIP:
            nc.vector.reciprocal(rep[d_row:d_row + 1, :, :],
                                 o[d_row:d_row + 1, :, :])
            dsc = dsc_pool.tile([1, CH], F32, tag="dsc", name="dsc")
            dma(out=dsc[0:1, :],
                in_=rep[d_row:d_row + 1, :, :].rearrange("p a b -> p (a b)"))
        else:
            nc.vector.tensor_copy(out=rep[d_row:d_row + 1, :, :],
                                  in_=o[d_row:d_row + 1, :, :])
            den0 = rep_pool.tile([1, CH], F32, tag="den0", name="den0")
            dma(out=den0[0:1, :],
                in_=rep[d_row:d_row + 1, :, :].rearrange("p a b -> p (a b)"))
            nc.vector.reciprocal_approx_fast(den0[0:1, :], den0[0:1, :])
            dsc = dsc_pool.tile([1, CH], F32, tag="dsc", name="dsc")
            dma(out=dsc[0:1, :], in_=den0[0:1, :])
        dma(out=rep[orng, :, :].rearrange("p a b -> p (a b)"),
            in_=_bcast_ap(dsc[0:1, :], 64))
        nc.vector.tensor_tensor(
            out=out_t[orng, jp, :].rearrange("p (a b) -> p a b", b=512),
            in0=o[orng, :, :], in1=rep[orng, :, :], op=OP.mult)
        if half == 1:
            nc.vector.tensor_scalar(out=out_t[:, jp, :], in0=out_t[:, jp, :],
                                    scalar1=bc(vb_nm, jp), scalar2=None,
                                    op0=OP.add)

    # ---- phase 2: self-attention ----
    for h in range(H):
        attn_head(h, ST, P, kt, qt, v1, ot, "vb")
    dump("ot", ot)
    free_v1()
    free_kt()
    free_qt()

    def proj_resid(w_t, in_t, res_t, out_t, b_nm, kj):
        """out_t[:,j,:] (f32) = w_t.T @ in_t + bias + res_t  (kj ptiles)."""
        for j in range(EJ):
            ps = ps_o.tile([P, 2, 512], F32, tag="po", name="ps_pr")
            for e in range(kj):
                for n in range(N2):
                    nc.tensor.matmul(
                        ps[:, n, :], lhsT=w_t[:, e, P * j:P * (j + 1)],
                        rhs=in_t[:, e, 512 * n:512 * (n + 1)],
                        start=(e == 0), stop=(e == kj - 1))
            nc.vector.scalar_tensor_tensor(
                out=out_t[:, j, :],
                in0=ps[:, :, :].rearrange("p a b -> p (a b)"),
                scalar=bc(b_nm, j), in1=res_t[:, j, :],
                op0=OP.add, op1=OP.add)

    def layernorm(src_t, out_t, g_nm, b_nm, side, mid=None):
        """LN over d (partitions x ptiles). src_t f32 [P,EJ,CH] (destroyed).

        `mid` (if given) is emitted after the reduction matmuls so
        independent PE work can overlap the DVE/ACT stats chain.
        """
        sq, free_sq = sb(f"sq_{g_nm}", [P, EJ, CH], BF16, side)
        st, free_st = sb(f"st_{g_nm}", [1, 3, CH], F32, side)
        # x**2 on the ACT engine (Square is in every activation table);
        # the mean-sum matmul reads the f32 source directly (4 cyc/row,
        # but saves the bf16 staging copy and its dependency).
        nc.scalar.activation(sq[:, :, :], src_t[:, :, :], AF.Square)
        sums_m = ps_o.tile([1, 2, 512], F32, tag="po", name="sums_m")
        sums_s = ps_s.tile([1, 2, 512], F32, tag="sc", name="sums_s")
        for n in range(N2):
            for e in range(EJ):
                nc.tensor.matmul(
                    sums_m[0:1, n, :], lhsT=ones_f[:, :],
                    rhs=src_t[:, e, 512 * n:512 * (n + 1)],
                    start=(e == 0), stop=(e == EJ - 1))
        for n in range(N2):
            for e in range(EJ):
                nc.tensor.matmul(
                    sums_s[0:1, n, :], lhsT=ones_col[:, :],
                    rhs=sq[:, e, 512 * n:512 * (n + 1)],
                    start=(e == 0), stop=(e == EJ - 1))
        if mid is not None:
            mid()
        # st slots: 0 = mean, 1 = var, 2 = rstd
        nc.vector.tensor_scalar(
            out=st[0:1, 0, :],
            in0=sums_m[0:1, :, :].rearrange("p a b -> p (a b)"),
            scalar1=1.0 / D, scalar2=None, op0=OP.mult)
        nc.vector.tensor_scalar(
            out=st[0:1, 1, :],
            in0=sums_s[0:1, :, :].rearrange("p a b -> p (a b)"),
            scalar1=1.0 / D, scalar2=None, op0=OP.mult)
        nc.vector.tensor_tensor(out=st[0:1, 2, :], in0=st[0:1, 0, :],
                                in1=st[0:1, 0, :], op=OP.mult)
        nc.vector.tensor_tensor(out=st[0:1, 1, :], in0=st[0:1, 1, :],
                                in1=st[0:1, 2, :], op=OP.subtract)
        # rstd = 1/sqrt(var + eps): Sqrt on ACT, fast reciprocal on DVE
        nc.scalar.activation(st[0:1, 1, :], st[0:1, 1, :], AF.Sqrt,
                             bias=eps_t[0:1, :])
        nc.vector.reciprocal_approx_fast(st[0:1, 2, :], st[0:1, 1, :])
        dsc = dsc_pool.tile([2, CH], F32, tag="dsc2", name="dsc2")
        dma(out=dsc[0:1, :], in_=st[0:1, 0, :])
        dma(out=dsc[1:2, :], in_=st[0:1, 2, :])
        rep_m = rep_pool.tile([P, 2, 512], F32, tag="rep", name="rep_m")
        rep_r = rep_pool.tile([P, 2, 512], F32, tag="rep", name="rep_r")
        dma(out=rep_m[:, :, :].rearrange("p a b -> p (a b)"),
            in_=_bcast_ap(dsc[0:1, :], P))
        dma(out=rep_r[:, :, :].rearrange("p a b -> p (a b)"),
            in_=_bcast_ap(dsc[1:2, :], P))
        for j in range(EJ):
            xv = src_t[:, j, :].rearrange("p (a b) -> p a b", b=512)
            nc.vector.tensor_tensor(out=xv, in0=xv, in1=rep_m[:, :, :],
                                    op=OP.subtract)
            nc.vector.tensor_tensor(out=xv, in0=xv, in1=rep_r[:, :, :],
                                    op=OP.mult)
            nc.vector.tensor_scalar(out=out_t[:, j, :], in0=src_t[:, j, :],
                                    scalar1=bc(g_nm, j), scalar2=bc(b_nm, j),
                                    op0=OP.mult, op1=OP.add)
        free_st()
        free_sq()

    # ---- phase 3: SA out-proj + residual + LN1 (CA k/v overlapped) ----
    x1, free_x1 = sb("x1", [P, EJ, CH], BF16, "left")
    xres, free_xres = sb("xres", [P, EJ, CH], F32, "left")
    proj_resid(w_sao, ot, xt_f32, xres, "saob", EJ)
    free_ot()
    free_xt_f32()

    # phase-4 activations (allocated now so LN1 temps stack above them)
    x2, free_x2 = sb("x2", [P, EJ, CH], BF16, "right")
    x2res, free_x2res = sb("x2res", [P, EJ, CH], F32, "right")
    oct_, free_oct = sb("oct", [P, EJ, CH], BF16, "right")
    qc, free_qc = sb("qc", [P, EJ, CH], BF16, "right")
    kc, free_kc = sb("kc", [P, EJ, 77], BF16, "right")
    vc1, free_vc1 = sb("vc1", [77, 1, (H // 2) * 160], BF16, "right")

    def ca_kv_proj():
        for j in range(EJ):
            ps = ps_o.tile([P, 2, 512], F32, tag="po", name="ps_ck")
            for e in range(CJ):
                nc.tensor.matmul(ps[:, 0, 0:77],
                                 lhsT=w_cak[:, e, P * j:P * (j + 1)],
                                 rhs=yt[:, e, :],
                                 start=(e == 0), stop=(e == CJ - 1))
            nc.vector.tensor_scalar(out=kc[:, j, :], in0=ps[:, 0, 0:77],
                                    scalar1=bc("cakb", j), scalar2=None,
                                    op0=OP.add)
        vc1h = vc1[:, :, :].rearrange("p t (pr c) -> p t pr c", c=160)
        nc.vector.memset(vc1h[:, :, :, 64:65], 1.0)
        nc.vector.memset(vc1h[:, :, :, 65:96], 0.0)
        psv = ps_o.tile([P, 2, 512], F32, tag="po", name="ps_cv")
        for e in range(CJ):
            nc.tensor.matmul(psv[0:77, 0, :], lhsT=yt[:, e, :],
                             rhs=w_cav[:, e, :], start=(e == 0),
                             stop=(e == CJ - 1))
        psvh = psv[0:77, 0, :].rearrange("p (pr two c) -> p pr two c",
                                         two=2, c=64)
        nc.vector.tensor_copy(out=vc1h[:, 0, :, 0:64], in_=psvh[:, :, 0, :])
        nc.vector.tensor_copy(out=vc1h[:, 0, :, 96:160], in_=psvh[:, :, 1, :])

    dump("xres_pre", xres) if False else None
    layernorm(xres, x1, "ln1g", "ln1b", "right", mid=ca_kv_proj)
    dump("x1", x1)
    free_xres()

    # ---- phase 4: cross-attention ----
    for j in range(EJ):
        ps = ps_o.tile([P, 2, 512], F32, tag="po", name="ps_cq")
        for e in range(EJ):
            for n in range(N2):
                nc.tensor.matmul(
                    ps[:, n, :], lhsT=w_caq[:, e, P * j:P * (j + 1)],
                    rhs=x1[:, e, 512 * n:512 * (n + 1)],
                    start=(e == 0), stop=(e == EJ - 1))
        nc.vector.tensor_scalar(
            out=qc[:, j, :],
            in0=ps[:, :, :].rearrange("p a b -> p (a b)"),
            scalar1=bc("caqb", j), scalar2=None, op0=OP.add)

    for h in range(H):
        attn_head(h, 1, 77, kc, qc, vc1, oct_, "cavb")
    dump("oct", oct_)
    free_vc1()
    free_kc()
    free_qc()

    proj_resid(w_cao, oct_, x1, x2res, "caob", EJ)
    free_oct()
    free_x1()

    # FFN weights load during LN2
    w_ff1, free_w_ff1 = sb("w_ff1", [P, EJ, FF], BF16, "left")
    dma(out=w_ff1[:, :, :], in_=w_ff1_d[:, :, :])
    w_ff2, free_w_ff2 = sb("w_ff2", [P, FJ, D], BF16, "left")
    dma(out=w_ff2[:, :, :], in_=w_ff2_d[:, :, :])

    layernorm(x2res, x2, "ln2g", "ln2b", "right")
    dump("x2", x2)
    free_x2res()

    # ---- phase 5: FFN ----
    x3res, free_x3res = sb("x3res", [P, EJ, CH], F32, "left")
    hbf, free_hbf = sb("hbf", [P, FJ, CH], BF16, "left")
    for f in range(FJ):
        ps = ps_o.tile([P, 2, 512], F32, tag="po", name="ps_f1")
        for e in range(EJ):
            for n in range(N2):
                nc.tensor.matmul(
                    ps[:, n, :], lhsT=w_ff1[:, e, P * f:P * (f + 1)],
                    rhs=x2[:, e, 512 * n:512 * (n + 1)],
                    start=(e == 0), stop=(e == EJ - 1))
        nc.scalar.activation(
            hbf[:, f, :].rearrange("p (a b) -> p a b", b=512), ps[:, :, :],
            GELU_AF, bias=bc("ffb1", f))
    proj_resid(w_ff2, hbf, x2, x3res, "ffb2", FJ)
    free_hbf()
    free_x2()
    layernorm(x3res, x3res, "ln3g", "ln3b", "left")
    for j in range(EJ):
        dma(out=out_d[:, j, :], in_=x3res[:, j, :])
    free_x3res()
    free_w_ff2()
    free_w_ff1()
    free_w_cao()
    free_w_cav()
    free_w_cak()
    free_w_caq()
    free_w_sao()
    free_yt()
    free_eps()
    free_ones_f()
    free_ones()
    free_bias()

    return names, out_name


_CACHE = {}


def _compiled():
    if "nc" not in _CACHE:
        nc = bacc.Bacc("TRN2", target_bir_lowering=False, debug=False)
        with tile.TileContext(nc) as tc:
            with tc.tile_pool(name="dram_io", bufs=1, space="DRAM") as dram:
                with ExitStack() as ctx:
                    names, out_name = build(ctx, tc, dram)
        nc.compile()
        _CACHE["nc"] = (nc, names, out_name)
    return _CACHE["nc"]


def make_in_maps(inputs, names):
    """Host-side sharding: full inputs -> 8 per-core in_maps."""
    bf = ml_dtypes.bfloat16
    f32 = np.float32
    x = np.asarray(inputs["x"], f32)
    y = np.asarray(inputs["y"], f32)
    w = {k: np.asarray(v, f32) for k, v in inputs.items()}

    bias = np.zeros((P, NBC), f32)
    for nm, src in [("qb", w["sa_in_b"][0:D]), ("kb", w["sa_in_b"][D:2 * D]),
                    ("vb", w["sa_in_b"][2 * D:3 * D]), ("saob", w["sa_out_b"]),
                    ("caqb", w["ca_q_b"]), ("cakb", w["ca_k_b"]),
                    ("cavb", w["ca_v_b"]), ("caob", w["ca_out_b"]),
                    ("ffb1", w["ff_b1"]), ("ffb2", w["ff_b2"]),
                    ("ln1g", w["ln1_g"]), ("ln1b", w["ln1_b"]),
                    ("ln2g", w["ln2_g"]), ("ln2b", w["ln2_b"]),
                    ("ln3g", w["ln3_g"]), ("ln3b", w["ln3_b"])]:
        c0, n = _BC[nm]
        bias[:, c0:c0 + n] = _bcol(src)

    wt = {
        "w_qkv": _pt(w["sa_in_w"]).astype(bf),
        "w_sao": _pt(w["sa_out_w"]).astype(bf),
        "w_caq": _pt(w["ca_q_w"]).astype(bf),
        "w_cak": _pt(w["ca_k_w"]).astype(bf),
        "w_cav": _pt(w["ca_v_w"]).astype(bf),
        "w_cao": _pt(w["ca_out_w"]).astype(bf),
        "w_ff1": _pt(w["ff_w1"]).astype(bf),
        "w_ff2": _pt(w["ff_w2"]).astype(bf),
        "bias": bias,
    }

    in_maps = []
    for c in range(NCORES):
        b, ch = c // 4, c % 4
        q0 = CH * ch
        # rotate tokens so the own chunk sits at columns 0:CH
        xtb = np.roll(_pt(x[b].T), -q0, axis=2)    # [128, EJ, S] f32
        m = {names[k]: v for k, v in wt.items()}
        m[names["xt_bf"]] = xtb.astype(bf)
        m[names["xt_f32"]] = np.ascontiguousarray(xtb[:, :, 0:CH])
        m[names["yt"]] = _pt(y[b].T).astype(bf)
        in_maps.append(m)
    return in_maps


def assemble(results, out_name):
    out = np.zeros((B, S, D), np.float32)
    for c in range(NCORES):
        b, ch = c // 4, c % 4
        arr = np.asarray(results[c][out_name])     # [128, EJ, CH]
        out[b, CH * ch:CH * (ch + 1), :] = (
            arr.transpose(1, 0, 2).reshape(D, CH).T)
    return out


def run(inputs, **spmd_kwargs):
    nc, names, out_name = _compiled()
    in_maps = make_in_maps(inputs, names)
    res = run_bass_kernel_spmd(nc, in_maps, core_ids=list(range(NCORES)),
                               **spmd_kwargs)
    return assemble(res.results, out_name), res


def kernel(**inputs):
    out, _ = run(inputs)
    return out


# revision 27
# speedup vs baseline: 1.1282x; 1.1282x over previous
"""Trainium2 Bass kernel: AttentionWithFeedForward (dense transformer block).

Sharding: 8 cores = (batch b = c//4) x (seq chunk of 1024 tokens = c%4).
Each core redundantly computes K/V over its full batch (no collectives),
Q/attention/FFN only for its own 1024-token chunk. The host rotates the
token axis per core so the own chunk is always columns 0:1024 (attention
is invariant to key order), keeping the device program identical across
cores.

Layout: all activations transposed [d_model, tok] ("ptile" layout
[128, d/128, tok]); host pre-transposes x/y and pre-casts weights to bf16.
Matmuls bf16 with fp32 PSUM accumulation.

Attention is a per-head software pipeline: per key-tile kk the PE emits
2 score matmuls (contraction d_head=64 on half the partitions), ACT emits
one Exp (the only ACT function in the attention phases -- no activation
table reloads), and the AV matmuls for kk-1 are emitted one iteration
behind so the PE never queues behind the exp of the same kk. Score PSUM
is double-buffered (2 tiles x 2 banks) and the AV accumulator pool is
double-buffered (2 x 2 banks) so head h+1's pipeline starts while head
h's normalize (DVE reciprocal + DMA broadcast + multiply) drains off the
critical path. Softmax denominators come from a ones-column packed into
V: even heads use a 65-wide lhsT (AV rows 0:64, denom row 64), odd heads
a 128-wide shifted view (AV rows 64:128, denom row 32) keeping DVE lane
alignment for the normalize multiply.

LayerNorm rstd = reciprocal(sqrt(var+eps)): Sqrt on ACT (its own table,
loaded once per LN) + fast approximate reciprocal on DVE.

SBUF is a two-sided stack allocator: frees must be LIFO per side, so big
tensors are placed left/right in nested lifetime order.
"""

from contextlib import ExitStack

import numpy as np
import ml_dtypes

import concourse.bass as bass
import concourse.tile as tile
from concourse import bacc, mybir
from concourse.bass_utils import run_bass_kernel_spmd

BF16 = mybir.dt.bfloat16
F32 = mybir.dt.float32
FP8 = mybir.dt.float8e4
I32 = mybir.dt.int32
AF = mybir.ActivationFunctionType
OP = mybir.AluOpType
PM = mybir.MatmulPerfMode

P = 128
D = 512          # d_embed
EJ = D // P      # 4 ptiles
DC = 768         # d_cross
CJ = DC // P     # 6
FF = 2048
FJ = FF // P     # 16
H = 8
DH = 64
S = 4096
ST = S // P      # 32 key tiles (full batch)
CH = 1024        # tokens per core
N2 = CH // 512   # 2 free-dim slices
B = 2
NCORES = 8
EPS = 1e-5
GELU_AF = AF.Gelu_apprx_tanh
DEBUG = False       # adds intermediate DRAM dumps (dev only)
# Schraudolph exp on DVE for every DVE_EXP_MOD-th SA key tile (0 = off):
# exp(x) ~ bitcast_f32(int32(A*x + B)), ~3% elem error that largely
# cancels in the softmax ratio; offloads the saturated ACT engine.
DVE_EXP_MOD = 3
FP8_AV = False   # fp8e4 V + DoubleRow AV matmuls in self-attention
SCH_A = float(2 ** 23 / np.log(2))
SCH_B = float(127 * 2 ** 23 - 0.043677 * 2 ** 23)
INLINE_AV = False   # emit AV right after exp (no one-behind pipelining)
# Attention denominators sit at partition base 32/64 where
# reciprocal_approx_fast silently returns garbage (custom-DVE op only
# works at partition base 0); False routes them through a partition-0
# bounce for the fast approx, True uses bit-exact reciprocal in place.
EXACT_RECIP = False

# bias_cols column layout; column j of a param holds param[128*j + p].
_BC = {}
_c = 0
for _nm, _n in [("qb", 4), ("kb", 4), ("vb", 4), ("saob", 4), ("caqb", 4),
                ("cakb", 4), ("cavb", 4), ("caob", 4), ("ffb1", 16),
                ("ffb2", 4), ("ln1g", 4), ("ln1b", 4), ("ln2g", 4),
                ("ln2b", 4), ("ln3g", 4), ("ln3b", 4)]:
    _BC[_nm] = (_c, _n)
    _c += _n
NBC = _c


def _pt(a):
    """[din, N] -> [128, din//128, N] ptile layout (partition-inner)."""
    din, n = a.shape
    return np.ascontiguousarray(a.reshape(din // P, P, n).transpose(1, 0, 2))


def _bcol(v):
    """[din] -> [128, din//128]."""
    return np.ascontiguousarray(v.reshape(-1, P).T)


def _bcast_ap(row_ap, nparts):
    """Broadcast a [1, N] DRAM AP across nparts partitions (step 0)."""
    return bass.AP(tensor=row_ap.tensor, offset=row_ap.offset,
                   ap=[[0, nparts]] + [list(d) for d in row_ap.ap[1:]])


def build(ctx, tc, dram):
    """Emit the full per-core program. Returns (names, out_name)."""
    nc = tc.nc
    names = {}

    def din(key, shape, dtype):
        t = dram.tile(shape, dtype, kind="ExternalInput", name=f"i_{key}")
        names[key] = t.name
        return t

    # ---- DRAM I/O ----
    xt_bf_d = din("xt_bf", [P, EJ, S], BF16)     # x[b].T rotated, bf16
    xt_f32_d = din("xt_f32", [P, EJ, CH], F32)   # own chunk (cols 0:CH), f32
    yt_d = din("yt", [P, CJ, 77], BF16)          # y[b].T
    w_qkv_d = din("w_qkv", [P, EJ, 3 * D], BF16)
    w_sao_d = din("w_sao", [P, EJ, D], BF16)
    w_caq_d = din("w_caq", [P, EJ, D], BF16)
    w_cak_d = din("w_cak", [P, CJ, D], BF16)
    w_cav_d = din("w_cav", [P, CJ, D], BF16)
    w_cao_d = din("w_cao", [P, EJ, D], BF16)
    w_ff1_d = din("w_ff1", [P, EJ, FF], BF16)
    w_ff2_d = din("w_ff2", [P, FJ, D], BF16)
    bias_d = din("bias", [P, NBC], F32)
    out_d = dram.tile([P, EJ, CH], F32, kind="ExternalOutput", name="o_out")
    out_name = out_d.name

    dma = nc.sync.dma_start

    def sb(key, shape, dtype, side):
        return tc.tile(shape, dtype, name=f"s_{key}", side=side)

    def dump(key, t):
        if not DEBUG:
            return
        dt_ = dram.tile(list(t.shape), t.dtype, kind="ExternalOutput",
                        name=f"dbg_{key}")
        names[f"dbg_{key}"] = dt_.name
        ap = tuple([slice(None)] * len(t.shape))
        dma(out=dt_[ap], in_=t[ap])

    # ---- pools (never popped before build end) ----
    ps_s = ctx.enter_context(tc.tile_pool(name="ps_s", bufs=2, space="PSUM"))
    ps_o = ctx.enter_context(tc.tile_pool(name="ps_o", bufs=2, space="PSUM"))
    et_pool = ctx.enter_context(
        tc.tile_pool(name="et_pool", bufs=3, side="left"))
    eti_pool = ctx.enter_context(
        tc.tile_pool(name="eti_pool", bufs=2, side="left"))
    rep_pool = ctx.enter_context(
        tc.tile_pool(name="rep_pool", bufs=2, side="left"))
    dsc_pool = ctx.enter_context(
        tc.tile_pool(name="dsc_pool", bufs=4, space="DRAM"))

    # ---- permanent small tiles (right-side bottom) ----
    bias_t, free_bias = sb("bias", [P, NBC], F32, "right")
    dma(out=bias_t[:, :], in_=bias_d[:, :])

    def bc(nm, j):
        c0, _n = _BC[nm]
        return bias_t[:, c0 + j:c0 + j + 1]

    ones_col, free_ones = sb("ones_col", [P, 1], BF16, "right")
    nc.vector.memset(ones_col[:, :], 1.0)
    ones_f, free_ones_f = sb("ones_f", [P, 1], F32, "right")
    nc.vector.memset(ones_f[:, :], 1.0)
    eps_t, free_eps = sb("eps", [1, 1], F32, "right")
    nc.vector.memset(eps_t[:, :], EPS)
    yt, free_yt = sb("yt", [P, CJ, 77], BF16, "right")
    dma(out=yt[:, :, :], in_=yt_d[:, :, :])

    # ---- weights prefetched early (right side, kept to end) ----
    w_sao, free_w_sao = sb("w_sao", [P, EJ, D], BF16, "right")
    w_caq, free_w_caq = sb("w_caq", [P, EJ, D], BF16, "right")
    w_cak, free_w_cak = sb("w_cak", [P, CJ, D], BF16, "right")
    w_cav, free_w_cav = sb("w_cav", [P, CJ, D], BF16, "right")
    w_cao, free_w_cao = sb("w_cao", [P, EJ, D], BF16, "right")

    # ---- left stack: phase-1/SA tensors ----
    qt, free_qt = sb("qt", [P, EJ, CH], BF16, "left")
    kt, free_kt = sb("kt", [P, EJ, S], BF16, "left")
    v1, free_v1 = sb("v1", [P, ST, (H // 2) * 160],
                     FP8 if FP8_AV else BF16, "left")
    xt_bf, free_xt_bf = sb("xt_bf", [P, EJ, S], BF16, "left")
    w_qkv, free_w_qkv = sb("w_qkv", [P, EJ, 3 * D], BF16, "left")
    dma(out=w_qkv[:, :, :], in_=w_qkv_d[:, :, :])
    for e in range(EJ):
        dma(out=xt_bf[:, e, :], in_=xt_bf_d[:, e, :])

    # residual-stream tensors
    xt_f32, free_xt_f32 = sb("xt_f32", [P, EJ, CH], F32, "right")
    dma(out=xt_f32[:, :, :], in_=xt_f32_d[:, :, :])
    ot, free_ot = sb("ot", [P, EJ, CH], BF16, "right")

    # remaining weight prefetches (behind the phase-1 inputs in the queue)
    dma(out=w_sao[:, :, :], in_=w_sao_d[:, :, :])
    dma(out=w_caq[:, :, :], in_=w_caq_d[:, :, :])
    dma(out=w_cak[:, :, :], in_=w_cak_d[:, :, :])
    dma(out=w_cav[:, :, :], in_=w_cav_d[:, :, :])
    dma(out=w_cao[:, :, :], in_=w_cao_d[:, :, :])

    v1h = v1[:, :, :].rearrange("p t (pr c) -> p t pr c", c=160)
    nc.vector.memset(v1h[:, :, :, 64:65], 1.0)
    nc.vector.memset(v1h[:, :, :, 65:96], 0.0)

    # ---- phase 1: QKV projections (transposed layout) ----
    # Loop order keeps the same lhsT for consecutive matmuls (weight reuse).
    for j in range(EJ):
        ps = ps_o.tile([P, 2, 512], F32, tag="po", name="ps_q")
        for e in range(EJ):
            for n in range(N2):
                nc.tensor.matmul(
                    ps[:, n, :], lhsT=w_qkv[:, e, P * j:P * (j + 1)],
                    rhs=xt_bf[:, e, 512 * n:512 * (n + 1)],
                    start=(e == 0), stop=(e == EJ - 1))
        nc.vector.tensor_scalar(
            out=qt[:, j, :],
            in0=ps[:, :, :].rearrange("p a b -> p (a b)"),
            scalar1=bc("qb", j), scalar2=None, op0=OP.add)
    for j in range(EJ):
        for g in range(S // CH):
            ps = ps_o.tile([P, 2, 512], F32, tag="po", name="ps_k")
            for e in range(EJ):
                for n in range(N2):
                    c0 = CH * g + 512 * n
                    nc.tensor.matmul(
                        ps[:, n, :], lhsT=w_qkv[:, e, D + P * j:D + P * (j + 1)],
                        rhs=xt_bf[:, e, c0:c0 + 512],
                        start=(e == 0), stop=(e == EJ - 1))
            nc.vector.tensor_scalar(
                out=kt[:, j, CH * g:CH * (g + 1)],
                in0=ps[:, :, :].rearrange("p a b -> p (a b)"),
                scalar1=bc("kb", j), scalar2=None, op0=OP.add)
    for t in range(ST):
        ps = ps_o.tile([P, 2, 512], F32, tag="po", name="ps_v")
        for e in range(EJ):
            nc.tensor.matmul(
                ps[:, 0, :], lhsT=xt_bf[:, e, P * t:P * (t + 1)],
                rhs=w_qkv[:, e, 2 * D:3 * D],
                start=(e == 0), stop=(e == EJ - 1))
        # V bias is applied after attention-normalize (per-partition there).
        # Both head-halves land in one strided copy: pair-block offsets
        # 0:64 (even) and 96:160 (odd) differ by a stride of 96.
        psh = ps[:, 0, :].rearrange("p (pr two c) -> p pr two c", two=2, c=64)
        v1t = v1[:, t, :]
        dst = bass.AP(tensor=v1t.tensor, offset=v1t.offset,
                      ap=[list(v1t.ap[0]), [160, H // 2], [96, 2], [1, 64]])
        nc.vector.tensor_copy(out=dst, in_=psh[:, :, :, :])
    dump("qt", qt)
    dump("kt", kt)
    dump("v1", v1)
    free_w_qkv()
    free_xt_bf()

    # ---- attention: per-head software pipeline ----
    def attn_head(h, kv_tiles, kp, kt_t, qt_t, v1_t, out_t, vb_nm):
        """One head: scores -> exp -> AV one iteration behind -> normalize.

        Even heads read AV rows 0:64 with the denominator at row 64; odd
        heads use the 128-wide shifted view of the packed [V|1] buffer so
        output lands on partitions 64:128 with the denominator at row 32.
        """
        jp, half = h // 2, h % 2
        dr = slice(DH * half, DH * (half + 1))
        o = ps_o.tile([P, 2, 512], F32, tag="po", name=f"o_h{half}")
        if half == 0:
            lhs_c0, om, d_row = 160 * jp, 65, 64
            orng = slice(0, 64)
        else:
            lhs_c0, om, d_row = 160 * jp + 32, 128, 32
            orng = slice(64, 128)

        def scores_exp(kk, et_dst):
            """Score matmuls for key-tile kk, then exp into et_dst
            ([kp, 2, 512] view, any dtype)."""
            sc = ps_s.tile([P, 2, 512], F32, tag="sc", name="sc")
            for n in range(N2):
                nc.tensor.matmul(
                    sc[0:kp, n, :],
                    lhsT=kt_t[dr, jp, P * kk:P * kk + kp],
                    rhs=qt_t[dr, jp, 512 * n:512 * (n + 1)],
                    start=True, stop=True)
            if DVE_EXP_MOD and kv_tiles > 1 and kk % DVE_EXP_MOD == (
                    DVE_EXP_MOD - 1):
                eti = eti_pool.tile([P, 2, 512], I32, tag="eti", name="eti")
                nc.vector.tensor_scalar(
                    out=eti[0:kp, :, :], in0=sc[0:kp, :, :],
                    scalar1=SCH_A * 0.125, scalar2=SCH_B,
                    op0=OP.mult, op1=OP.add)
                nc.vector.tensor_copy(out=et_dst,
                                      in_=eti[0:kp, :, :].bitcast(F32))
            else:
                nc.scalar.activation(et_dst, sc[0:kp, :, :], AF.Exp,
                                     scale=0.125)

        if kv_tiles > 1 and FP8_AV:
            # fp8 AV with DoubleRow: two key-tiles per matmul, et tiles
            # hold an exp PAIR [kp, kk-parity, n, 512] in fp8e4.
            npair = kv_tiles // 2

            def av8(pp, et):
                lhs = v1_t[0:kp, 2 * pp:2 * pp + 2, lhs_c0:lhs_c0 + om]
                for n in range(N2):
                    nc.tensor.matmul(o[0:om, n, :], lhsT=lhs,
                                     rhs=et[0:kp, :, n, :],
                                     start=(pp == 0), stop=(pp == npair - 1),
                                     perf_mode=PM.DoubleRow)

            prev = None
            for pp in range(npair):
                et = et_pool.tile([P, 2, 2, 512], FP8, tag="et", name="et")
                scores_exp(2 * pp, et[0:kp, 0, :, :])
                if prev is not None:
                    av8(pp - 1, prev)
                scores_exp(2 * pp + 1, et[0:kp, 1, :, :])
                prev = et
            av8(npair - 1, prev)
        else:
            def av(kk, et):
                lhs = v1_t[0:kp, kk, lhs_c0:lhs_c0 + om]
                for n in range(N2):
                    nc.tensor.matmul(o[0:om, n, :], lhsT=lhs,
                                     rhs=et[0:kp, n, :],
                                     start=(kk == 0),
                                     stop=(kk == kv_tiles - 1))

            prev = None
            for kk in range(kv_tiles):
                et = et_pool.tile([P, 2, 512], BF16, tag="et", name="et")
                scores_exp(kk, et[0:kp, :, :])
                if prev is not None:
                    av(kk - 1, prev)
                prev = et
            av(kv_tiles - 1, prev)

        # normalize: rep rows = 1/denom broadcast, out = O*rep.
        # The denominator row sits at partition 32/64 where the fast
        # approximate reciprocal (custom DVE op) misbehaves, so: copy the
        # PSUM row to SBUF, DMA-shift it to a partition-0 tile, take the
        # fast reciprocal there, then bounce through DRAM to broadcast.
        rep = rep_pool.tile([P, 2, 512], F32, tag="rep", name="rep")
        if EXACT_RECIP:
            nc.vector.reciprocal(rep[d_row:d_row + 1, :, :],
                                 o[d_row:d_row + 1, :, :])
            dsc = dsc_pool.tile([1, CH], F32, tag="dsc", name="dsc")
            dma(out=dsc[0:1, :],
                in_=rep[d_row:d_row + 1, :, :].rearrange("p a b -> p (a b)"))
        else:
            nc.vector.tensor_copy(out=rep[d_row:d_row + 1, :, :],
                                  in_=o[d_row:d_row + 1, :, :])
            den0 = rep_pool.tile([1, CH], F32, tag="den0", name="den0")
            dma(out=den0[0:1, :],
                in_=rep[d_row:d_row + 1, :, :].rearrange("p a b -> p (a b)"))
            nc.vector.reciprocal_approx_fast(den0[0:1, :], den0[0:1, :])
            dsc = dsc_pool.tile([1, CH], F32, tag="dsc", name="dsc")
            dma(out=dsc[0:1, :], in_=den0[0:1, :])
        dma(out=rep[orng, :, :].rearrange("p a b -> p (a b)"),
            in_=_bcast_ap(dsc[0:1, :], 64))
        nc.vector.tensor_tensor(
            out=out_t[orng, jp, :].rearrange("p (a b) -> p a b", b=512),
            in0=o[orng, :, :], in1=rep[orng, :, :], op=OP.mult)
        if half == 1:
            nc.vector.tensor_scalar(out=out_t[:, jp, :], in0=out_t[:, jp, :],
                                    scalar1=bc(vb_nm, jp), scalar2=None,
                                    op0=OP.add)

    # ---- phase 2: self-attention ----
    for h in range(H):
        attn_head(h, ST, P, kt, qt, v1, ot, "vb")
    dump("ot", ot)
    free_v1()
    free_kt()
    free_qt()

    def proj_resid(w_t, in_t, res_t, out_t, b_nm, kj):
        """out_t[:,j,:] (f32) = w_t.T @ in_t + bias + res_t  (kj ptiles)."""
        for j in range(EJ):
            ps = ps_o.tile([P, 2, 512], F32, tag="po", name="ps_pr")
            for e in range(kj):
                for n in range(N2):
                    nc.tensor.matmul(
                        ps[:, n, :], lhsT=w_t[:, e, P * j:P * (j + 1)],
                        rhs=in_t[:, e, 512 * n:512 * (n + 1)],
                        start=(e == 0), stop=(e == kj - 1))
            nc.vector.scalar_tensor_tensor(
                out=out_t[:, j, :],
                in0=ps[:, :, :].rearrange("p a b -> p (a b)"),
                scalar=bc(b_nm, j), in1=res_t[:, j, :],
                op0=OP.add, op1=OP.add)

    def layernorm(src_t, out_t, g_nm, b_nm, side, mid=None):
        """LN over d (partitions x ptiles). src_t f32 [P,EJ,CH] (destroyed).

        `mid` (if given) is emitted after the reduction matmuls so
        independent PE work can overlap the DVE/ACT stats chain.
        """
        sq, free_sq = sb(f"sq_{g_nm}", [P, EJ, CH], BF16, side)
        st, free_st = sb(f"st_{g_nm}", [1, 3, CH], F32, side)
        # x**2 on the ACT engine (Square is in every activation table);
        # the mean-sum matmul reads the f32 source directly (4 cyc/row,
        # but saves the bf16 staging copy and its dependency).
        nc.scalar.activation(sq[:, :, :], src_t[:, :, :], AF.Square)
        sums_m = ps_o.tile([1, 2, 512], F32, tag="po", name="sums_m")
        sums_s = ps_s.tile([1, 2, 512], F32, tag="sc", name="sums_s")
        for n in range(N2):
            for e in range(EJ):
                nc.tensor.matmul(
                    sums_m[0:1, n, :], lhsT=ones_f[:, :],
                    rhs=src_t[:, e, 512 * n:512 * (n + 1)],
                    start=(e == 0), stop=(e == EJ - 1))
        for n in range(N2):
            for e in range(EJ):
                nc.tensor.matmul(
                    sums_s[0:1, n, :], lhsT=ones_col[:, :],
                    rhs=sq[:, e, 512 * n:512 * (n + 1)],
                    start=(e == 0), stop=(e == EJ - 1))
        if mid is not None:
            mid()
        # st slots: 0 = mean, 1 = var, 2 = rstd
        nc.vector.tensor_scalar(
            out=st[0:1, 0, :],
            in0=sums_m[0:1, :, :].rearrange("p a b -> p (a b)"),
            scalar1=1.0 / D, scalar2=None, op0=OP.mult)
        nc.vector.tensor_scalar(
            out=st[0:1, 1, :],
            in0=sums_s[0:1, :, :].rearrange("p a b -> p (a b)"),
            scalar1=1.0 / D, scalar2=None, op0=OP.mult)
        nc.vector.tensor_tensor(out=st[0:1, 2, :], in0=st[0:1, 0, :],
                                in1=st[0:1, 0, :], op=OP.mult)
        nc.vector.tensor_tensor(out=st[0:1, 1, :], in0=st[0:1, 1, :],
                                in1=st[0:1, 2, :], op=OP.subtract)
        # rstd = 1/sqrt(var + eps): Sqrt on ACT, fast reciprocal on DVE
        nc.scalar.activation(st[0:1, 1, :], st[0:1, 1, :], AF.Sqrt,
                             bias=eps_t[0:1, :])
        nc.vector.reciprocal_approx_fast(st[0:1, 2, :], st[0:1, 1, :])
        dsc = dsc_pool.tile([2, CH], F32, tag="dsc2", name="dsc2")
        dma(out=dsc[0:1, :], in_=st[0:1, 0, :])
        dma(out=dsc[1:2, :], in_=st[0:1, 2, :])
        rep_m = rep_pool.tile([P, 2, 512], F32, tag="rep", name="rep_m")
        rep_r = rep_pool.tile([P, 2, 512], F32, tag="rep", name="rep_r")
        dma(out=rep_m[:, :, :].rearrange("p a b -> p (a b)"),
            in_=_bcast_ap(dsc[0:1, :], P))
        dma(out=rep_r[:, :, :].rearrange("p a b -> p (a b)"),
            in_=_bcast_ap(dsc[1:2, :], P))
        for j in range(EJ):
            xv = src_t[:, j, :].rearrange("p (a b) -> p a b", b=512)
            nc.vector.tensor_tensor(out=xv, in0=xv, in1=rep_m[:, :, :],
                                    op=OP.subtract)
            nc.vector.tensor_tensor(out=xv, in0=xv, in1=rep_r[:, :, :],
                                    op=OP.mult)
            nc.vector.tensor_scalar(out=out_t[:, j, :], in0=src_t[:, j, :],
                                    scalar1=bc(g_nm, j), scalar2=bc(b_nm, j),
                                    op0=OP.mult, op1=OP.add)
        free_st()
        free_sq()

    # ---- phase 3: SA out-proj + residual + LN1 (CA k/v overlapped) ----
    x1, free_x1 = sb("x1", [P, EJ, CH], BF16, "left")
    xres, free_xres = sb("xres", [P, EJ, CH], F32, "left")
    proj_resid(w_sao, ot, xt_f32, xres, "saob", EJ)
    free_ot()
    free_xt_f32()

    # phase-4 activations (allocated now so LN1 temps stack above them)
    x2, free_x2 = sb("x2", [P, EJ, CH], BF16, "right")
    x2res, free_x2res = sb("x2res", [P, EJ, CH], F32, "right")
    oct_, free_oct = sb("oct", [P, EJ, CH], BF16, "right")
    qc, free_qc = sb("qc", [P, EJ, CH], BF16, "right")
    kc, free_kc = sb("kc", [P, EJ, 77], BF16, "right")
    vc1, free_vc1 = sb("vc1", [77, 1, (H // 2) * 160], BF16, "right")

    def ca_kv_proj():
        for j in range(EJ):
            ps = ps_o.tile([P, 2, 512], F32, tag="po", name="ps_ck")
            for e in range(CJ):
                nc.tensor.matmul(ps[:, 0, 0:77],
                                 lhsT=w_cak[:, e, P * j:P * (j + 1)],
                                 rhs=yt[:, e, :],
                                 start=(e == 0), stop=(e == CJ - 1))
            nc.vector.tensor_scalar(out=kc[:, j, :], in0=ps[:, 0, 0:77],
                                    scalar1=bc("cakb", j), scalar2=None,
                                    op0=OP.add)
        vc1h = vc1[:, :, :].rearrange("p t (pr c) -> p t pr c", c=160)
        nc.vector.memset(vc1h[:, :, :, 64:65], 1.0)
        nc.vector.memset(vc1h[:, :, :, 65:96], 0.0)
        psv = ps_o.tile([P, 2, 512], F32, tag="po", name="ps_cv")
        for e in range(CJ):
            nc.tensor.matmul(psv[0:77, 0, :], lhsT=yt[:, e, :],
                             rhs=w_cav[:, e, :], start=(e == 0),
                             stop=(e == CJ - 1))
        psvh = psv[0:77, 0, :].rearrange("p (pr two c) -> p pr two c",
                                         two=2, c=64)
        nc.vector.tensor_copy(out=vc1h[:, 0, :, 0:64], in_=psvh[:, :, 0, :])
        nc.vector.tensor_copy(out=vc1h[:, 0, :, 96:160], in_=psvh[:, :, 1, :])

    dump("xres_pre", xres) if False else None
    layernorm(xres, x1, "ln1g", "ln1b", "right", mid=ca_kv_proj)
    dump("x1", x1)
    free_xres()

    # ---- phase 4: cross-attention ----
    for j in range(EJ):
        ps = ps_o.tile([P, 2, 512], F32, tag="po", name="ps_cq")
        for e in range(EJ):
            for n in range(N2):
                nc.tensor.matmul(
                    ps[:, n, :], lhsT=w_caq[:, e, P * j:P * (j + 1)],
                    rhs=x1[:, e, 512 * n:512 * (n + 1)],
                    start=(e == 0), stop=(e == EJ - 1))
        nc.vector.tensor_scalar(
            out=qc[:, j, :],
            in0=ps[:, :, :].rearrange("p a b -> p (a b)"),
            scalar1=bc("caqb", j), scalar2=None, op0=OP.add)

    for h in range(H):
        attn_head(h, 1, 77, kc, qc, vc1, oct_, "cavb")
    dump("oct", oct_)
    free_vc1()
    free_kc()
    free_qc()

    proj_resid(w_cao, oct_, x1, x2res, "caob", EJ)
    free_oct()
    free_x1()

    # FFN weights load during LN2
    w_ff1, free_w_ff1 = sb("w_ff1", [P, EJ, FF], BF16, "left")
    dma(out=w_ff1[:, :, :], in_=w_ff1_d[:, :, :])
    w_ff2, free_w_ff2 = sb("w_ff2", [P, FJ, D], BF16, "left")
    dma(out=w_ff2[:, :, :], in_=w_ff2_d[:, :, :])

    layernorm(x2res, x2, "ln2g", "ln2b", "right")
    dump("x2", x2)
    free_x2res()

    # ---- phase 5: FFN ----
    x3res, free_x3res = sb("x3res", [P, EJ, CH], F32, "left")
    hbf, free_hbf = sb("hbf", [P, FJ, CH], BF16, "left")
    for f in range(FJ):
        ps = ps_o.tile([P, 2, 512], F32, tag="po", name="ps_f1")
        for e in range(EJ):
            for n in range(N2):
                nc.tensor.matmul(
                    ps[:, n, :], lhsT=w_ff1[:, e, P * f:P * (f + 1)],
                    rhs=x2[:, e, 512 * n:512 * (n + 1)],
                    start=(e == 0), stop=(e == EJ - 1))
        nc.scalar.activation(
            hbf[:, f, :].rearrange("p (a b) -> p a b", b=512), ps[:, :, :],
            GELU_AF, bias=bc("ffb1", f))
    proj_resid(w_ff2, hbf, x2, x3res, "ffb2", FJ)
    free_hbf()
    free_x2()
    layernorm(x3res, x3res, "ln3g", "ln3b", "left")
    for j in range(EJ):
        dma(out=out_d[:, j, :], in_=x3res[:, j, :])
    free_x3res()
    free_w_ff2()
    free_w_ff1()
    free_w_cao()
    free_w_cav()
    free_w_cak()
    free_w_caq()
    free_w_sao()
    free_yt()
    free_eps()
    free_ones_f()
    free_ones()
    free_bias()

    return names, out_name


_CACHE = {}


def _compiled():
    if "nc" not in _CACHE:
        nc = bacc.Bacc("TRN2", target_bir_lowering=False, debug=False)
        with tile.TileContext(nc) as tc:
            with tc.tile_pool(name="dram_io", bufs=1, space="DRAM") as dram:
                with ExitStack() as ctx:
                    names, out_name = build(ctx, tc, dram)
        nc.compile()
        _CACHE["nc"] = (nc, names, out_name)
    return _CACHE["nc"]


def make_in_maps(inputs, names):
    """Host-side sharding: full inputs -> 8 per-core in_maps."""
    bf = ml_dtypes.bfloat16
    f32 = np.float32
    x = np.asarray(inputs["x"], f32)
    y = np.asarray(inputs["y"], f32)
    w = {k: np.asarray(v, f32) for k, v in inputs.items()}

    bias = np.zeros((P, NBC), f32)
    for nm, src in [("qb", w["sa_in_b"][0:D]), ("kb", w["sa_in_b"][D:2 * D]),
                    ("vb", w["sa_in_b"][2 * D:3 * D]), ("saob", w["sa_out_b"]),
                    ("caqb", w["ca_q_b"]), ("cakb", w["ca_k_b"]),
                    ("cavb", w["ca_v_b"]), ("caob", w["ca_out_b"]),
                    ("ffb1", w["ff_b1"]), ("ffb2", w["ff_b2"]),
                    ("ln1g", w["ln1_g"]), ("ln1b", w["ln1_b"]),
                    ("ln2g", w["ln2_g"]), ("ln2b", w["ln2_b"]),
                    ("ln3g", w["ln3_g"]), ("ln3b", w["ln3_b"])]:
        c0, n = _BC[nm]
        bias[:, c0:c0 + n] = _bcol(src)

    wt = {
        "w_qkv": _pt(w["sa_in_w"]).astype(bf),
        "w_sao": _pt(w["sa_out_w"]).astype(bf),
        "w_caq": _pt(w["ca_q_w"]).astype(bf),
        "w_cak": _pt(w["ca_k_w"]).astype(bf),
        "w_cav": _pt(w["ca_v_w"]).astype(bf),
        "w_cao": _pt(w["ca_out_w"]).astype(bf),
        "w_ff1": _pt(w["ff_w1"]).astype(bf),
        "w_ff2": _pt(w["ff_w2"]).astype(bf),
        "bias": bias,
    }

    in_maps = []
    for c in range(NCORES):
        b, ch = c // 4, c % 4
        q0 = CH * ch
        # rotate tokens so the own chunk sits at columns 0:CH
        xtb = np.roll(_pt(x[b].T), -q0, axis=2)    # [128, EJ, S] f32
        m = {names[k]: v for k, v in wt.items()}
        m[names["xt_bf"]] = xtb.astype(bf)
        m[names["xt_f32"]] = np.ascontiguousarray(xtb[:, :, 0:CH])
        m[names["yt"]] = _pt(y[b].T).astype(bf)
        in_maps.append(m)
    return in_maps


def assemble(results, out_name):
    out = np.zeros((B, S, D), np.float32)
    for c in range(NCORES):
        b, ch = c // 4, c % 4
        arr = np.asarray(results[c][out_name])     # [128, EJ, CH]
        out[b, CH * ch:CH * (ch + 1), :] = (
            arr.transpose(1, 0, 2).reshape(D, CH).T)
    return out


def run(inputs, **spmd_kwargs):
    nc, names, out_name = _compiled()
    in_maps = make_in_maps(inputs, names)
    res = run_bass_kernel_spmd(nc, in_maps, core_ids=list(range(NCORES)),
                               **spmd_kwargs)
    return assemble(res.results, out_name), res


def kernel(**inputs):
    out, _ = run(inputs)
    return out
